# revision 2
# baseline (speedup 1.0000x reference)
"""Trainium2 Bass kernel for nn_DenoiserBlock (B=2, L=2048, D=1024, H=16,
F=4096). 8 cores = 2 (batch) x 4 (query-slice of 512); each core redundantly
computes K/V for its batch element and attends its 512-query slice.

Fast path (shared torus_scale): fp8e4m3 DoubleRow matmuls for all large GEMMs
(QKV/out-proj hi-lo/FFN hi-lo splits), torus/mask bias injected into the
scores PSUM by a DoubleRow identity-matmul from an fp8e5m2 bias tensor,
1024-wide Exp straight from PSUM to fp8e5m2 attention weights, DMA-engine
transposes, batched LayerNorm scalar chains, and engine-balanced PSUM
evacuations. Falls back to an all-bf16 kernel when torus_scale is per-head.
"""

import sys

sys.path.insert(0, "/opt/trn_rl_repo")

import numpy as np
import ml_dtypes

import concourse.bacc as bacc
import concourse.mybir as mybir
from concourse import tile, masks
from concourse.bass_utils import run_bass_kernel_spmd

F32 = mybir.dt.float32
BF16 = mybir.dt.bfloat16
F32R = mybir.dt.float32r
E4 = mybir.dt.float8e4
E5 = mybir.dt.float8e5
AX = mybir.AxisListType
OP = mybir.AluOpType
ACT = mybir.ActivationFunctionType
PM = mybir.MatmulPerfMode

B, L, D, H, F = 2, 2048, 1024, 16, 4096
HD = D // H
QS = 512
EPS = 1e-5
NLT = L // 128   # 16
NDT = D // 128   # 8
NQT = QS // 128  # 4
NFT = F // 128   # 32


def build_fast():
    nc = bacc.Bacc("TRN2", target_bir_lowering=False, debug=False, num_devices=8)

    d_x = nc.dram_tensor("x_full", [L, D], F32, kind="ExternalInput")
    d_wq = nc.dram_tensor("wq8", [128, NDT, D], E4, kind="ExternalInput")
    d_wk = nc.dram_tensor("wk8", [128, NDT, D], E4, kind="ExternalInput")
    d_wv = nc.dram_tensor("wv8", [128, NDT + 2, D], E4, kind="ExternalInput")
    d_wo = nc.dram_tensor("wout8", [128, NDT, D], E4, kind="ExternalInput")
    d_wol = nc.dram_tensor("wout8l", [128, NDT, D], E4, kind="ExternalInput")
    d_w1 = nc.dram_tensor("w18", [128, NDT, F], E4, kind="ExternalInput")
    d_w1l = nc.dram_tensor("w18l", [128, NDT, F], E4, kind="ExternalInput")
    d_w2 = nc.dram_tensor("w28", [128, NFT, D], E4, kind="ExternalInput")
    d_w2l = nc.dram_tensor("w28l", [128, NFT, D], E4, kind="ExternalInput")
    d_bT = nc.dram_tensor("biasT8", [128, NLT, QS], E5, kind="ExternalInput")
    d_bq = nc.dram_tensor("biasq", [128, NDT], F32, kind="ExternalInput")
    d_bk = nc.dram_tensor("biask", [128, NDT], F32, kind="ExternalInput")
    d_b2 = nc.dram_tensor("bias2r", [128, D], F32, kind="ExternalInput")
    d_b1 = nc.dram_tensor("b1sb", [128, NFT], F32, kind="ExternalInput")
    d_sc = nc.dram_tensor("scal", [128, 8], F32, kind="ExternalInput")
    d_y = nc.dram_tensor("y", [QS, D], F32, kind="ExternalOutput")

    # The host rolls x so this core's query slice is always rows [0, QS);
    # softmax over keys is permutation-invariant, and the bias tensor is
    # built in the rolled frame, so results are identical.
    q0 = 0

    with tile.TileContext(nc) as tc:
        with (
            tc.tile_pool(name="const", bufs=1) as cpool,
            tc.tile_pool(name="dp", bufs=1) as dpool,
        ):
            scal = cpool.tile([128, 8], F32, tag="scal")
            biasq = cpool.tile([128, NDT], F32, tag="biasq")
            biask = cpool.tile([128, NDT], F32, tag="biask")
            bias2r = cpool.tile([128, D], F32, tag="bias2r")
            b1sb = cpool.tile([128, NFT], F32, tag="b1sb")
            epsc = cpool.tile([128, 1], F32, tag="epsc")
            ident = cpool.tile([128, 128], BF16, tag="ident")
            iz = cpool.tile([128, 2, 128], E4, tag="iz")
            zi = cpool.tile([128, 2, 128], E4, tag="zi")
            nc.sync.dma_start(scal[:], d_sc[:, :])
            nc.sync.dma_start(biasq[:], d_bq[:, :])
            nc.sync.dma_start(biask[:], d_bk[:, :])
            nc.sync.dma_start(bias2r[:], d_b2[:, :])
            nc.sync.dma_start(b1sb[:], d_b1[:, :])
            nc.vector.memset(epsc[:], EPS)
            masks.make_identity(nc, ident[:])
            nc.vector.memset(iz[:].rearrange("p a b -> p (a b)"), 0.0)
            nc.vector.memset(zi[:].rearrange("p a b -> p (a b)"), 0.0)
            nc.vector.tensor_scalar(iz[:, 0, :], ident[:], 1.0 / 64, None,
                                    op0=OP.mult)
            nc.vector.tensor_scalar(zi[:, 1, :], ident[:], 1.0 / 64, None,
                                    op0=OP.mult)

            # Live C..E
            outT8 = [dpool.tile([128, 2, QS], E4, tag=f"oT{i}", name=f"oT{i}")
                     for i in range(4)]
            x2 = [dpool.tile([128, D], F32, tag=f"x2{i}", name=f"x2{i}")
                  for i in range(NQT)]
            h2T = dpool.tile([128, NDT, QS], BF16, tag="h2T")
            h2T8 = dpool.tile([128, NDT, QS], E4, tag="h2T8")
            h2T8l = dpool.tile([128, NDT, QS], E4, tag="h2T8l")
            w1hi = dpool.tile([128, NDT, F], E4, tag="w1hi")
            wos = dpool.tile([128, NDT, D], E4, tag="wos")

            def ln_stats(pool, xt, sb, col):
                """One [128, D] fp32 tile -> bf16 copy; row sums into column
                `col` of the batched stat tiles sb = (s1a, s2a, ...)."""
                s1a, s2a = sb[0], sb[1]
                xb = pool.tile([128, D], BF16, tag="xb", name="xb", bufs=8)
                sq = pool.tile([128, D], BF16, tag="sq", name="sq", bufs=1)
                nc.scalar.activation(xb[:], xt[:], ACT.Identity,
                                     accum_out=s1a[:, col:col + 1])
                nc.vector.tensor_tensor_reduce(
                    out=sq[:], in0=xt[:], in1=xt[:], scale=1.0,
                    scalar=0.0, op0=OP.mult, op1=OP.add,
                    accum_out=s2a[:, col:col + 1])
                return xb

            def ln_smalls(sb, lo, hi):
                """Batched scalar chain over stat columns [lo, hi): 6 ops
                total, two engine crossings."""
                s1a, s2a, mua, vara, rsta, nmra = sb
                c = slice(lo, hi)
                nc.vector.tensor_scalar(mua[:, c], s1a[:, c], 1.0 / D, None,
                                        op0=OP.mult)
                nc.vector.tensor_tensor(vara[:, c], mua[:, c], mua[:, c],
                                        op=OP.mult)
                nc.vector.scalar_tensor_tensor(
                    vara[:, c], s2a[:, c], 1.0 / D, vara[:, c],
                    op0=OP.mult, op1=OP.subtract)
                nc.scalar.activation(vara[:, c], vara[:, c], ACT.Sqrt,
                                     bias=epsc[:])
                nc.vector.reciprocal(rsta[:, c], vara[:, c])
                nc.vector.scalar_tensor_tensor(
                    nmra[:, c], mua[:, c], -1.0, rsta[:, c],
                    op0=OP.mult, op1=OP.mult)

            def ln_norm(xb, sb, col, hb, flip):
                mua, rsta, nmra = sb[2], sb[4], sb[5]
                if flip:
                    nc.scalar.activation(hb[:], xb[:], ACT.Identity,
                                         bias=nmra[:, col:col + 1],
                                         scale=rsta[:, col:col + 1])
                else:
                    nc.vector.tensor_scalar(hb[:], xb[:],
                                            mua[:, col:col + 1],
                                            rsta[:, col:col + 1],
                                            op0=OP.subtract, op1=OP.mult)

            def ln_statbank(pool, n, tag):
                return tuple(pool.tile([128, n], F32, tag=f"{tag}{i}",
                                       name=f"{tag}{i}") for i in range(6))

            with tc.tile_pool(name="attn", bufs=1) as apool:
                kT = [apool.tile([128, L], E4, tag=f"kT{i}", name=f"kT{i}")
                      for i in range(NDT)]
                qT = [apool.tile([128, QS], E4, tag=f"qT{i}", name=f"qT{i}")
                      for i in range(NDT)]
                v8 = apool.tile([128, NLT, H * (HD + 1)], E4, tag="v8")
                bT8 = apool.tile([128, NLT, QS], E5, tag="bT8")
                nc.sync.dma_start(bT8[:, :, :], d_bT[:, :, :])

                # ---------- Phase A: LN1 + DMA transpose -> hT8 ----------
                with (
                    tc.tile_pool(name="phAB", bufs=1) as bpool,
                    tc.tile_pool(name="psB", bufs=1, space="PSUM") as psA,
                ):
                    hT8 = bpool.tile([128, NDT + 2, L], E4, tag="hT8")
                    nc.vector.memset(
                        hT8[:, NDT:NDT + 2, :].rearrange("p a b -> p (a b)"),
                        0.0)
                    nc.vector.memset(hT8[0:1, NDT, :], 1.0)
                    wqs = bpool.tile([128, NDT + 2, D], E4, tag="wb",
                                     name="wq", bufs=2)
                    wks = bpool.tile([128, NDT + 2, D], E4, tag="wb",
                                     name="wk", bufs=2)
                    wvs = bpool.tile([128, NDT + 2, D], E4, tag="wb",
                                     name="wv", bufs=2)
                    asb = ln_statbank(bpool, NLT, "als")
                    xbs = {}
                    xtiles = {}

                    def emit_dma(lt):
                        xt = bpool.tile([128, D], F32, tag="xt", name="xt",
                                        bufs=3)
                        nc.sync.dma_start(xt[:],
                                          d_x[lt * 128:(lt + 1) * 128, :])
                        xtiles[lt] = xt

                    def emit_finish(lt):
                        even = (lt % 2 == 0)
                        hb = bpool.tile([128, D], BF16, tag="hb", name="hb",
                                        bufs=3)
                        ln_norm(xbs[lt], asb, lt, hb, flip=even)
                        st = bpool.tile([128, NDT, 128], BF16, tag="st",
                                        name="st", bufs=3)
                        nc.sync.dma_start_transpose(st[:], hb[:])
                        if even:
                            nc.gpsimd.tensor_copy(
                                hT8[:, 0:NDT, lt * 128:(lt + 1) * 128], st[:])
                        else:
                            nc.vector.tensor_copy(
                                hT8[:, 0:NDT, lt * 128:(lt + 1) * 128], st[:])

                    for lt in range(8):
                        emit_dma(lt)
                        xbs[lt] = ln_stats(bpool, xtiles[lt], asb, lt)
                    nc.sync.dma_start(wvs[:], d_wv[:, :, :])
                    nc.sync.dma_start(wqs[:, 0:NDT, :], d_wq[:, :, :])
                    for lt in range(8, NLT):
                        emit_dma(lt)
                    ln_smalls(asb, 0, 8)
                    for lt in range(8):
                        emit_finish(lt)
                    for lt in range(8, NLT):
                        xbs[lt] = ln_stats(bpool, xtiles[lt], asb, lt)
                    ln_smalls(asb, 8, NLT)
                    for lt in range(8, NLT):
                        emit_finish(lt)

                    # ---------- Phase B: QKV projections (fp8 DR) ----------

                    for kt in range(NLT):
                        v3 = v8[:, kt, :].rearrange("p (h c) -> p h c",
                                                    c=HD + 1)
                        for half in range(2):
                            pv = psA.tile([128, QS], F32, tag="mm", name="pv",
                                          bufs=4)
                            for j in range(5):
                                nc.tensor.matmul(
                                    pv[:], hT8[:, 2 * j:2 * j + 2,
                                               kt * 128:(kt + 1) * 128],
                                    wvs[:, 2 * j:2 * j + 2,
                                        half * QS:(half + 1) * QS],
                                    perf_mode=PM.DoubleRow,
                                    start=(j == 0), stop=(j == 4))
                            if (kt + half) % 2 == 0:
                                nc.scalar.activation(
                                    v3[:, half * 8:(half + 1) * 8, 0:HD],
                                    pv[:], ACT.Identity, bias=0.0,
                                    scale=scal[:, 2:3])
                            else:
                                nc.vector.tensor_scalar(
                                    v3[:, half * 8:(half + 1) * 8, 0:HD],
                                    pv[:], scal[:, 2:3], None, op0=OP.mult)
                        nc.vector.memset(v3[:, :, HD:HD + 1], 1.0)

                    for i in range(NDT):
                        pq = psA.tile([128, QS], F32, tag="mm", name="pq",
                                      bufs=4)
                        for j in range(4):
                            nc.tensor.matmul(
                                pq[:], wqs[:, 2 * j:2 * j + 2,
                                           i * 128:(i + 1) * 128],
                                hT8[:, 2 * j:2 * j + 2, q0:q0 + QS],
                                perf_mode=PM.DoubleRow,
                                start=(j == 0), stop=(j == 3))
                        if i % 2 == 0:
                            nc.scalar.activation(qT[i][:], pq[:], ACT.Identity,
                                                 bias=biasq[:, i:i + 1],
                                                 scale=scal[:, 0:1])
                        else:
                            nc.vector.tensor_scalar(
                                qT[i][:], pq[:], scal[:, 0:1],
                                biasq[:, i:i + 1], op0=OP.mult, op1=OP.add)
                    nc.sync.dma_start(wks[:, 0:NDT, :], d_wk[:, :, :])
                    for i in range(NDT):
                        for col in range(L // QS):
                            pk = psA.tile([128, QS], F32, tag="mm", name="pk",
                                          bufs=4)
                            for j in range(4):
                                nc.tensor.matmul(
                                    pk[:], wks[:, 2 * j:2 * j + 2,
                                               i * 128:(i + 1) * 128],
                                    hT8[:, 2 * j:2 * j + 2,
                                        col * QS:(col + 1) * QS],
                                    perf_mode=PM.DoubleRow,
                                    start=(j == 0), stop=(j == 3))
                            if (i + col) % 2 == 0:
                                nc.scalar.activation(
                                    kT[i][:, col * QS:(col + 1) * QS], pk[:],
                                    ACT.Identity, bias=biask[:, i:i + 1],
                                    scale=scal[:, 1:2])
                            else:
                                nc.vector.tensor_scalar(
                                    kT[i][:, col * QS:(col + 1) * QS], pk[:],
                                    scal[:, 1:2], biask[:, i:i + 1],
                                    op0=OP.mult, op1=OP.add)
                # ---------- Phase C: attention ----------
                with (
                    tc.tile_pool(name="phC", bufs=1) as cwork,
                    tc.tile_pool(name="psC", bufs=1, space="PSUM") as psC,
                ):
                    nc.sync.dma_start(wos[:], d_wo[:, :, :])
                    for ch in range(4):
                        nc.sync.dma_start(
                            w1hi[:, :, ch * 1024:(ch + 1) * 1024],
                            d_w1[:, :, ch * 1024:(ch + 1) * 1024])
                    for h in range(H):
                        t, ho = h // 2, (h % 2) * 64
                        po = psC.tile([65, QS], F32, tag="po", name="po",
                                      bufs=2)
                        for p in range(NLT // 2):
                            ps3 = psC.tile([128, 2, QS], F32, tag="ps3",
                                           name="ps3", bufs=3)
                            for j2 in range(2):
                                kt = 2 * p + j2
                                nc.tensor.matmul(
                                    ps3[:, j2, :], (iz if j2 == 0 else zi)[:],
                                    bT8[:, 2 * p:2 * p + 2, :],
                                    perf_mode=PM.DoubleRow,
                                    start=True, stop=False)
                                nc.tensor.matmul(
                                    ps3[:, j2, :],
                                    kT[t][ho:ho + 64,
                                          kt * 128:(kt + 1) * 128],
                                    qT[t][ho:ho + 64, :],
                                    start=False, stop=True,
                                    skip_group_check=True)
                            pm = cwork.tile([128, 2, QS], E5, tag="pm",
                                            name="pm", bufs=4)
                            nc.scalar.activation(pm[:], ps3[:], ACT.Exp)
                            nc.tensor.matmul(
                                po[:], v8[:, 2 * p:2 * p + 2,
                                          h * 65:h * 65 + 65],
                                pm[:], perf_mode=PM.DoubleRow,
                                start=(p == 0), stop=(p == NLT // 2 - 1))
                        rsum = cwork.tile([1, QS], F32, tag="rr", name="rsum",
                                          bufs=4)
                        nc.vector.tensor_scalar(rsum[:], po[64:65, :], 1e-30,
                                                None, op0=OP.add)
                        recip = cwork.tile([1, QS], F32, tag="rr",
                                           name="recip", bufs=4)
                        nc.vector.reciprocal(recip[:], rsum[:])
                        rbs = cwork.tile([64, QS], F32, tag="rbs", name="rbs",
                                         bufs=2)
                        nc.gpsimd.partition_broadcast(rbs[:], recip[:])
                        nc.vector.tensor_tensor(
                            outT8[h // 4][ho:ho + 64, (h // 2) % 2, :],
                            po[0:64, :], rbs[:], op=OP.mult)

            # ---------- Phase D+E: out proj + LN2 + FFN ----------
            with tc.tile_pool(name="phDE", bufs=1) as dwork:
                wol = dwork.tile([128, NDT, D], E4, tag="wol")
                nc.sync.dma_start(wol[:], d_wol[:, :, :])
                xrs = []
                for qt in range(NQT):
                    xr = dwork.tile([128, D], F32, tag="xr", name="xr",
                                    bufs=4)
                    nc.sync.dma_start(
                        xr[:], d_x[qt * 128:(qt + 1) * 128, :])
                    xrs.append(xr)
                w1lo = dwork.tile([128, NDT, F], E4, tag="w1lo")
                for c in range(4):
                    nc.sync.dma_start(w1lo[:, :, c * 1024:(c + 1) * 1024],
                                      d_w1l[:, :, c * 1024:(c + 1) * 1024])
                aT8 = dwork.tile([128, NFT, QS], E4, tag="aT8")

                with tc.tile_pool(name="psD", bufs=1, space="PSUM") as psDD:
                    dsb = ln_statbank(dwork, NQT, "dls")
                    d_xb = {}
                    for qt in range(NQT):
                        xr = xrs[qt]
                        for half in range(2):
                            p2 = psDD.tile([128, QS], F32, tag="mm",
                                           name="p2", bufs=4)
                            for p in range(4):
                                nc.tensor.matmul(
                                    p2[:],
                                    outT8[p][:, :, qt * 128:(qt + 1) * 128],
                                    wos[:, 2 * p:2 * p + 2,
                                        half * QS:(half + 1) * QS],
                                    perf_mode=PM.DoubleRow,
                                    start=(p == 0), stop=False)
                            for p in range(4):
                                nc.tensor.matmul(
                                    p2[:],
                                    outT8[p][:, :, qt * 128:(qt + 1) * 128],
                                    wol[:, 2 * p:2 * p + 2,
                                        half * QS:(half + 1) * QS],
                                    perf_mode=PM.DoubleRow,
                                    start=False, stop=(p == 3))
                            nc.vector.scalar_tensor_tensor(
                                x2[qt][:, half * QS:(half + 1) * QS], p2[:],
                                scal[:, 3:4],
                                xr[:, half * QS:(half + 1) * QS],
                                op0=OP.mult, op1=OP.add)
                        d_xb[qt] = ln_stats(dwork, x2[qt], dsb, qt)

                    ln_smalls(dsb, 0, NQT)
                    for qt in range(NQT):
                        even = (qt % 2 == 0)
                        hb2 = dwork.tile([128, D], BF16, tag="hb2",
                                         name="hb2", bufs=2)
                        ln_norm(d_xb[qt], dsb, qt, hb2, flip=even)
                        nc.sync.dma_start_transpose(
                            h2T[:, :, qt * 128:(qt + 1) * 128], hb2[:])
                        nc.vector.tensor_copy(
                            h2T8[:, :, qt * 128:(qt + 1) * 128],
                            h2T[:, :, qt * 128:(qt + 1) * 128])
                        nc.vector.tensor_tensor(
                            h2T8l[:, :, qt * 128:(qt + 1) * 128],
                            h2T[:, :, qt * 128:(qt + 1) * 128],
                            h2T8[:, :, qt * 128:(qt + 1) * 128],
                            op=OP.subtract)
                        nc.vector.tensor_tensor(x2[qt][:], x2[qt][:],
                                                bias2r[:], op=OP.add)

                with tc.tile_pool(name="psE1", bufs=1, space="PSUM") as psE1:
                    for fp in range(NFT // 2):
                        pa = psE1.tile([128, 2, QS], F32, tag="pa", name="pa",
                                       bufs=2)
                        for sub in range(2):
                            ft = 2 * fp + sub
                            for j in range(4):
                                nc.tensor.matmul(
                                    pa[:, sub, :],
                                    w1hi[:, 2 * j:2 * j + 2,
                                         ft * 128:(ft + 1) * 128],
                                    h2T8[:, 2 * j:2 * j + 2, :],
                                    perf_mode=PM.DoubleRow,
                                    start=(j == 0), stop=False)
                            for j in range(4):
                                nc.tensor.matmul(
                                    pa[:, sub, :],
                                    w1lo[:, 2 * j:2 * j + 2,
                                         ft * 128:(ft + 1) * 128],
                                    h2T8[:, 2 * j:2 * j + 2, :],
                                    perf_mode=PM.DoubleRow,
                                    start=False, stop=False)
                            for j in range(4):
                                nc.tensor.matmul(
                                    pa[:, sub, :],
                                    w1hi[:, 2 * j:2 * j + 2,
                                         ft * 128:(ft + 1) * 128],
                                    h2T8l[:, 2 * j:2 * j + 2, :],
                                    perf_mode=PM.DoubleRow,
                                    start=False, stop=(j == 3))
                            nc.scalar.activation(
                                aT8[:, ft, :], pa[:, sub, :],
                                ACT.Gelu_apprx_tanh,
                                bias=b1sb[:, ft:ft + 1], scale=scal[:, 4:5])

                with tc.tile_pool(name="psE2", bufs=1, space="PSUM") as psE2:
                    py = [psE2.tile([128, 2, QS], F32, tag=f"py{q}",
                                    name=f"py{q}") for q in range(NQT)]
                    for fq in range(NFT // 4):
                        c2h = dwork.tile([128, 4, D], E4, tag="c2h",
                                         name="c2h", bufs=2)
                        c2l = dwork.tile([128, 4, D], E4, tag="c2l",
                                         name="c2l", bufs=2)
                        nc.sync.dma_start(c2h[:],
                                          d_w2[:, 4 * fq:4 * fq + 4, :])
                        nc.sync.dma_start(c2l[:],
                                          d_w2l[:, 4 * fq:4 * fq + 4, :])
                        for sub in range(2):
                            fp = 2 * fq + sub
                            for qt in range(NQT):
                                for half in range(2):
                                    nc.tensor.matmul(
                                        py[qt][:, half, :],
                                        aT8[:, 2 * fp:2 * fp + 2,
                                            qt * 128:(qt + 1) * 128],
                                        c2h[:, 2 * sub:2 * sub + 2,
                                            half * QS:(half + 1) * QS],
                                        perf_mode=PM.DoubleRow,
                                        start=(fp == 0), stop=False)
                                    nc.tensor.matmul(
                                        py[qt][:, half, :],
                                        aT8[:, 2 * fp:2 * fp + 2,
                                            qt * 128:(qt + 1) * 128],
                                        c2l[:, 2 * sub:2 * sub + 2,
                                            half * QS:(half + 1) * QS],
                                        perf_mode=PM.DoubleRow,
                                        start=False,
                                        stop=(fp == NFT // 2 - 1))
                    for qt in range(NQT):
                        ysb = dwork.tile([128, D], F32, tag="ysb",
                                         name="ysb", bufs=2)
                        nc.vector.scalar_tensor_tensor(
                            ysb[:], py[qt][:].rearrange("p a b -> p (a b)"),
                            scal[:, 5:6], x2[qt][:], op0=OP.mult, op1=OP.add)
                        nc.sync.dma_start(d_y[qt * 128:(qt + 1) * 128, :],
                                          ysb[:])

    nc.compile()
    return nc


def _q8(a, dt=ml_dtypes.float8_e4m3):
    lim = 440.0 if dt == ml_dtypes.float8_e4m3 else 57000.0
    return np.clip(np.asarray(a, np.float32), -lim, lim).astype(dt)


def _q8_hilo(a):
    hi = _q8(a)
    lo = _q8(np.asarray(a, np.float64) - hi.astype(np.float64))
    return hi, lo


def _pick_scale(w):
    rms = float(np.sqrt(np.mean(np.asarray(w, np.float64) ** 2))) + 1e-30
    return float(2.0 ** np.clip(np.round(np.log2(8.0 / rms)), -8, 10))


def _wlay(w, ktiles):
    """[K, N] -> [128, ktiles, N] with K = ktiles*128, [p, j, n] = w[j*128+p, n]."""
    K, N = w.shape
    assert K == ktiles * 128
    return np.ascontiguousarray(w.reshape(ktiles, 128, N).transpose(1, 0, 2))


def make_inputs(x, torus_dist, time_emb, mask, ln1_g, ln1_b, Wqkv, Wout,
                torus_scale, ln2_g, ln2_b, W1, b1, W2, b2, Wt, bt, gelu_tanh):
    """Host-side prep. Returns per-core input maps (8 cores)."""
    rep = lambda v: np.ascontiguousarray(
        np.tile(np.asarray(v, np.float32)[None, :], (128, 1)))

    tp = (gelu_tanh(time_emb) @ np.asarray(Wt, np.float64)
          + np.asarray(bt, np.float64))
    scale, shift = tp[:, :D], tp[:, D:]
    g_eff = (np.asarray(ln1_g, np.float64)[None, :] * (1.0 + scale))
    b_eff = (np.asarray(ln1_b, np.float64)[None, :] * (1.0 + scale) + shift)

    Wq_r = np.asarray(Wqkv[:, 0:D], np.float64) / np.sqrt(HD)
    Wk_r = np.asarray(Wqkv[:, D:2 * D], np.float64)
    Wv_r = np.asarray(Wqkv[:, 2 * D:3 * D], np.float64)

    g2 = np.asarray(ln2_g, np.float64)
    w1g = g2[:, None] * np.asarray(W1, np.float64)
    S1 = _pick_scale(w1g)
    w1hi_, w1lo_ = _q8_hilo(w1g * S1)
    w18 = _wlay(w1hi_, NDT)
    w18l = _wlay(w1lo_, NDT)
    b1_eff = (np.asarray(b1, np.float64)
              + np.asarray(ln2_b, np.float64) @ np.asarray(W1, np.float64))
    b1sb = np.ascontiguousarray(
        np.asarray(b1_eff, np.float32).reshape(NFT, 128).T)

    S2 = _pick_scale(W2)
    w2hi_, w2lo_ = _q8_hilo(np.asarray(W2, np.float64) * S2)
    w28 = _wlay(w2hi_, NFT)
    w28l = _wlay(w2lo_, NFT)
    So = _pick_scale(Wout)
    wohi_, wolo_ = _q8_hilo(np.asarray(Wout, np.float64) * So)
    wo8 = _wlay(wohi_, NDT)
    wo8l = _wlay(wolo_, NDT)
    bias2r = rep(b2)

    sc_arr = np.asarray(torus_scale, np.float32)
    sc0 = float(sc_arr[0])

    in_maps = []
    for c in range(8):
        b_, qs_ = c // 4, c % 4
        roll = qs_ * QS
        # roll x so this core's query slice is rows [0, QS)
        xb = np.roll(np.asarray(x[b_], np.float32), -roll, axis=0)
        ge, be = g_eff[b_], b_eff[b_]
        wq_s = ge[:, None] * Wq_r
        wk_s = ge[:, None] * Wk_r
        wv_s = ge[:, None] * Wv_r
        Sq, Sk, Sv = (_pick_scale(wq_s), _pick_scale(wk_s),
                      _pick_scale(wv_s))
        wq8 = _wlay(_q8(wq_s * Sq), NDT)
        wk8 = _wlay(_q8(wk_s * Sk), NDT)
        wv8_core = _wlay(_q8(wv_s * Sv), NDT)
        wv8 = np.zeros((128, NDT + 2, D), ml_dtypes.float8_e4m3)
        wv8[:, :NDT, :] = wv8_core
        wv8[0, NDT, :] = _q8((be @ Wv_r) * Sv)
        bq = (be @ Wq_r).astype(np.float32)
        bk = (be @ Wk_r).astype(np.float32)
        bv = (be @ Wv_r).astype(np.float32)

        # bias tensor: [keys, queries] in ROLLED frame:
        # rolled key k -> orig key (k+roll)%L; query q -> orig row q+roll
        km = np.where(np.asarray(mask[b_]), 0.0, -88.0).astype(np.float64)
        tor = np.asarray(torus_dist[0], np.float64)  # [orig_q, orig_k]
        qrows = (np.arange(QS) + roll) % L
        torT = tor[qrows, :].T  # [orig_k, q]
        biasT = (np.roll(km, -roll)[:, None]
                 - sc0 * np.roll(torT, -roll, axis=0)) * 64.0
        biasT8 = np.ascontiguousarray(
            _q8(biasT, ml_dtypes.float8_e5m2).reshape(NLT, 128, QS)
            .transpose(1, 0, 2))

        scal = np.zeros((128, 8), np.float32)
        scal[:, 0] = 1.0 / Sq
        scal[:, 1] = 1.0 / Sk
        scal[:, 2] = 8.0 / Sv
        scal[:, 3] = 1.0 / (8.0 * So)
        scal[:, 4] = 1.0 / S1
        scal[:, 5] = 1.0 / S2

        in_maps.append({
            "x_full": np.ascontiguousarray(xb),
            "wq8": wq8, "wk8": wk8, "wv8": wv8,
            "wout8": wo8, "wout8l": wo8l,
            "w18": w18, "w18l": w18l, "w28": w28, "w28l": w28l,
            "biasT8": biasT8,
            "biasq": np.ascontiguousarray(bq.reshape(NDT, 128).T),
            "biask": np.ascontiguousarray(bk.reshape(NDT, 128).T),
            "bias2r": bias2r, "b1sb": b1sb,
            "scal": scal,
        })
    return in_maps


_CACHED = {}

NC_PER_B = 4


def _build_fallback(shared_mask=False):
    nc = bacc.Bacc("TRN2", target_bir_lowering=False, debug=False, num_devices=8)

    d_x = nc.dram_tensor("x_full", [L, D], F32, kind="ExternalInput")
    d_xres = nc.dram_tensor("x_res", [QS, D], F32, kind="ExternalInput")
    if shared_mask:
        d_expm = nc.dram_tensor("expm", [L, QS], BF16, kind="ExternalInput")
    else:
        d_expm = nc.dram_tensor("expm", [H, L, QS], BF16, kind="ExternalInput")
    d_wq = nc.dram_tensor("wq", [D, D], BF16, kind="ExternalInput")
    d_wk = nc.dram_tensor("wk", [D, D], BF16, kind="ExternalInput")
    d_wv = nc.dram_tensor("wv", [D, D], BF16, kind="ExternalInput")
    d_wout = nc.dram_tensor("wout", [D, D], BF16, kind="ExternalInput")
    d_w1t = nc.dram_tensor("w1t", [32, 8, 128, 128], BF16, kind="ExternalInput")
    d_w2 = nc.dram_tensor("w2", [F, D], BF16, kind="ExternalInput")
    d_bias2r = nc.dram_tensor("bias2r", [128, D], F32, kind="ExternalInput")
    d_biask = nc.dram_tensor("biask", [128, 8], F32, kind="ExternalInput")
    d_biasq = nc.dram_tensor("biasq", [128, 8], F32, kind="ExternalInput")
    d_bvrep = nc.dram_tensor("bvrep", [128, D], F32, kind="ExternalInput")
    d_b1sb = nc.dram_tensor("b1sb", [128, 32], F32, kind="ExternalInput")
    d_y = nc.dram_tensor("y", [QS, D], F32, kind="ExternalOutput")

    NLT = L // 128
    NDT = D // 128
    NQT = QS // 128
    NFT = F // 128

    with tile.TileContext(nc) as tc:
        with (
            tc.tile_pool(name="const", bufs=1) as cpool,
            tc.tile_pool(name="mid", bufs=1) as mpool,
            tc.tile_pool(name="psum", bufs=1, space="PSUM") as pspool,
        ):
            b1sb = cpool.tile([128, 32], F32, tag="b1sb")
            ident = cpool.tile([128, 128], BF16, tag="ident")
            epsc = cpool.tile([128, 1], F32, tag="epsc")
            biask = cpool.tile([128, 8], F32, tag="biask")
            biasq = cpool.tile([128, 8], F32, tag="biasq")
            bvrep = cpool.tile([128, D], F32, tag="bvrep")
            nc.sync.dma_start(b1sb[:], d_b1sb[:, :])
            nc.sync.dma_start(biask[:], d_biask[:, :])
            nc.sync.dma_start(biasq[:], d_biasq[:, :])
            nc.sync.dma_start(bvrep[:], d_bvrep[:, :])
            masks.make_identity(nc, ident[:])
            nc.vector.memset(epsc[:], EPS)

            outT = [mpool.tile([128, QS], BF16, tag=f"outT{i}", name=f"outT{i}")
                    for i in range(NDT)]
            x2 = [mpool.tile([128, D], F32, tag=f"x2{i}", name=f"x2{i}")
                  for i in range(NQT)]
            h2T = [mpool.tile([128, QS], BF16, tag=f"h2T{i}", name=f"h2T{i}")
                   for i in range(NDT)]

            def layer_norm_tile(pool, pspool, xt, hT_tiles, col0):
                """Normalize one [128, D] tile (no gain/bias - folded into the
                consuming weights host-side) -> bf16 transposed blocks into
                hT_tiles[j][:, col0:col0+128]."""
                s1 = pool.tile([128, 1], F32, tag="lns", name="s1", bufs=21)
                s2 = pool.tile([128, 1], F32, tag="lns", name="s2", bufs=21)
                mu = pool.tile([128, 1], F32, tag="lns", name="mu", bufs=21)
                msq = pool.tile([128, 1], F32, tag="lns", name="msq", bufs=21)
                var = pool.tile([128, 1], F32, tag="lns", name="var", bufs=21)
                std = pool.tile([128, 1], F32, tag="lns", name="std", bufs=21)
                rstd = pool.tile([128, 1], F32, tag="lns", name="rstd", bufs=21)
                sq = pool.tile([128, D], F32, tag="xc", name="sq")
                hb = pool.tile([128, D], BF16, tag="hb", name="hb")
                nc.vector.tensor_reduce(s1[:], xt[:], axis=AX.X, op=OP.add)
                nc.scalar.activation(sq[:], xt[:], ACT.Square, accum_out=s2[:])
                nc.scalar.mul(mu[:], s1[:], 1.0 / D)
                nc.vector.tensor_tensor(msq[:], mu[:], mu[:], op=OP.mult)
                nc.vector.scalar_tensor_tensor(
                    var[:], s2[:], 1.0 / D, msq[:], op0=OP.mult, op1=OP.subtract)
                nc.scalar.activation(std[:], var[:], ACT.Sqrt, bias=epsc[:])
                nc.vector.reciprocal(rstd[:], std[:])
                nc.vector.tensor_scalar(hb[:], xt[:], mu[:], rstd[:],
                                        op0=OP.subtract, op1=OP.mult)
                for j in range(NDT):
                    pt = pspool.tile([128, 128], BF16, tag="trp", name="trp", bufs=2)
                    nc.tensor.transpose(pt[:], hb[:, j * 128:(j + 1) * 128], ident[:])
                    if j % 2 == 0:
                        nc.scalar.copy(hT_tiles[j][:, col0:col0 + 128], pt[:])
                    else:
                        nc.vector.tensor_copy(hT_tiles[j][:, col0:col0 + 128], pt[:])

            with tc.tile_pool(name="attn", bufs=1) as atpool:
                kT = [atpool.tile([128, L], BF16, tag=f"kT{i}", name=f"kT{i}")
                      for i in range(NDT)]
                vv = [atpool.tile([128, H * (HD + 1)], BF16, tag=f"v{i}", name=f"v{i}")
                      for i in range(NLT)]
                qT = [atpool.tile([128, QS], BF16, tag=f"qT{i}", name=f"qT{i}")
                      for i in range(NDT)]

                # ---- Phase A ----
                with tc.tile_pool(name="hTp", bufs=1) as hpool:
                    hT = [hpool.tile([128, L], BF16, tag=f"hT{i}", name=f"hT{i}")
                          for i in range(NDT)]
                    hresT = [hpool.tile([128, QS], BF16, tag=f"hrT{i}", name=f"hrT{i}")
                             for i in range(NDT)]
                    with tc.tile_pool(name="phA", bufs=5) as apool:
                        for lt in range(NLT):
                            xt = apool.tile([128, D], F32, tag="xt", name="xt", bufs=4)
                            nc.sync.dma_start(xt[:], d_x[lt * 128:(lt + 1) * 128, :])
                            layer_norm_tile(apool, pspool, xt, hT, lt * 128)
                        for rt in range(NQT):
                            xt = apool.tile([128, D], F32, tag="xt", name="xt", bufs=4)
                            nc.sync.dma_start(xt[:], d_xres[rt * 128:(rt + 1) * 128, :])
                            layer_norm_tile(apool, pspool, xt, hresT, rt * 128)

                    # ---- Phase B ----
                    with tc.tile_pool(name="wtsQ", bufs=1) as wqpool:
                        wq = [wqpool.tile([128, D], BF16, tag=f"wq{i}", name=f"wq{i}")
                              for i in range(NDT)]
                        for i in range(NDT):
                            nc.sync.dma_start(wq[i][:], d_wq[i * 128:(i + 1) * 128, :])
                        for i in range(NDT):
                            pq = pspool.tile([128, 512], F32, tag="mm", name="pq", bufs=4)
                            for dt_ in range(NDT):
                                nc.tensor.matmul(
                                    pq[:], wq[dt_][:, i * 128:(i + 1) * 128],
                                    hresT[dt_][:],
                                    start=(dt_ == 0), stop=(dt_ == NDT - 1))
                            nc.vector.tensor_scalar(qT[i][:], pq[:], biasq[:, i:i + 1],
                                                    None, op0=OP.add)

                    with tc.tile_pool(name="wtsK", bufs=1) as wkpool:
                        wk = [wkpool.tile([128, D], BF16, tag=f"wk{i}", name=f"wk{i}")
                              for i in range(NDT)]
                        for i in range(NDT):
                            nc.sync.dma_start(wk[i][:], d_wk[i * 128:(i + 1) * 128, :])
                        for i in range(NDT):
                            for ncol in range(L // 512):
                                pk = pspool.tile([128, 512], F32, tag="mm", name="pk", bufs=4)
                                for dt_ in range(NDT):
                                    nc.tensor.matmul(
                                        pk[:], wk[dt_][:, i * 128:(i + 1) * 128],
                                        hT[dt_][:, ncol * 512:(ncol + 1) * 512],
                                        start=(dt_ == 0), stop=(dt_ == NDT - 1))
                                nc.vector.tensor_scalar(
                                    kT[i][:, ncol * 512:(ncol + 1) * 512], pk[:],
                                    biask[:, i:i + 1], None, op0=OP.add)

                    with tc.tile_pool(name="wtsV", bufs=1) as wvpool:
                        wv = [wvpool.tile([128, D], BF16, tag=f"wv{i}", name=f"wv{i}")
                              for i in range(NDT)]
                        for i in range(NDT):
                            nc.sync.dma_start(wv[i][:], d_wv[i * 128:(i + 1) * 128, :])
                        for lt in range(NLT):
                            v3 = vv[lt][:].rearrange("p (h c) -> p h c", c=HD + 1)
                            for half in range(2):
                                pv = pspool.tile([128, 512], F32, tag="mm", name="pv", bufs=4)
                                for dt_ in range(NDT):
                                    nc.tensor.matmul(
                                        pv[:], hT[dt_][:, lt * 128:(lt + 1) * 128],
                                        wv[dt_][:, half * 512:(half + 1) * 512],
                                        start=(dt_ == 0), stop=(dt_ == NDT - 1))
                                nc.vector.tensor_tensor(
                                    v3[:, half * 8:(half + 1) * 8, 0:HD], pv[:],
                                    bvrep[:, half * 512:(half + 1) * 512], op=OP.add)
                            nc.vector.memset(v3[:, :, HD:HD + 1], 1.0)

                # ---- Phase C ----
                with (
                    tc.tile_pool(name="phC", bufs=8) as cwork,
                    tc.tile_pool(name="mres", bufs=1) as mpool_c,
                ):
                    mres = None
                    if shared_mask:
                        mres = [mpool_c.tile([128, QS], BF16, tag=f"mr{i}",
                                             name=f"mr{i}") for i in range(NLT)]
                        for kt in range(NLT):
                            nc.sync.dma_start(
                                mres[kt][:], d_expm[kt * 128:(kt + 1) * 128, :])
                    for hp in range(H // 2):
                        ht = hp
                        pos = [pspool.tile([65, 512], F32, tag="acc",
                                           name=f"po{par}", bufs=2) for par in range(2)]
                        for kt in range(NLT):
                            for par in range(2):
                                h, ho = 2 * hp + par, par * 64
                                if shared_mask:
                                    mt = mres[kt]
                                else:
                                    mt = cwork.tile([128, 512], BF16, tag="mt",
                                                    name="mt")
                                    nc.sync.dma_start(
                                        mt[:], d_expm[h, kt * 128:(kt + 1) * 128, :])
                                ps = pspool.tile([128, 512], F32, tag="mm",
                                                 name="ps", bufs=4)
                                nc.tensor.matmul(
                                    ps[:], kT[ht][ho:ho + 64, kt * 128:(kt + 1) * 128],
                                    qT[ht][ho:ho + 64, :], start=True, stop=True)
                                pb = cwork.tile([128, 512], BF16, tag="pb", name="pb")
                                nc.scalar.activation(pb[:], ps[:], ACT.Exp)
                                pm = cwork.tile([128, 512], BF16, tag="pm", name="pm")
                                nc.vector.tensor_tensor(pm[:], pb[:], mt[:],
                                                        op=OP.mult)
                                v3 = vv[kt][:].rearrange("p (h c) -> p h c", c=HD + 1)
                                nc.tensor.matmul(
                                    pos[par][:], v3[:, h, :], pm[:],
                                    start=(kt == 0), stop=(kt == NLT - 1))
                        for par in range(2):
                            ho = par * 64
                            rsum = cwork.tile([1, 512], F32, tag="recip",
                                              name="rsum")
                            nc.vector.tensor_scalar(rsum[:], pos[par][64:65, :],
                                                    1e-30, None, op0=OP.add)
                            recip = cwork.tile([1, 512], F32, tag="recip",
                                               name="recip")
                            nc.vector.reciprocal(recip[:], rsum[:])
                            rbs = cwork.tile([64, 512], F32, tag="rbs", name="rbs")
                            nc.gpsimd.partition_broadcast(rbs[:], recip[:])
                            nc.vector.tensor_tensor(
                                outT[ht][ho:ho + 64, :], pos[par][0:64, :], rbs[:],
                                op=OP.mult)

                # ---- Phase D ----
                with (
                    tc.tile_pool(name="phD", bufs=6) as dwork,
                    tc.tile_pool(name="phD_w", bufs=1) as dwpool,
                ):
                    bias2r = dwpool.tile([128, D], F32, tag="bias2r")
                    nc.sync.dma_start(bias2r[:], d_bias2r[:, :])
                    wo = [dwpool.tile([128, D], BF16, tag=f"wo{i}", name=f"wo{i}")
                          for i in range(NDT)]
                    for i in range(NDT):
                        nc.sync.dma_start(wo[i][:], d_wout[i * 128:(i + 1) * 128, :])
                    xr = [dwpool.tile([128, D], F32, tag=f"xr{i}", name=f"xr{i}")
                          for i in range(NQT)]
                    for i in range(NQT):
                        nc.sync.dma_start(xr[i][:], d_xres[i * 128:(i + 1) * 128, :])
                    for qt in range(NQT):
                        for half in range(2):
                            p2 = pspool.tile([128, 512], F32, tag="mm", name="p2", bufs=4)
                            for dt_ in range(NDT):
                                nc.tensor.matmul(
                                    p2[:], outT[dt_][:, qt * 128:(qt + 1) * 128],
                                    wo[dt_][:, half * 512:(half + 1) * 512],
                                    start=(dt_ == 0), stop=(dt_ == NDT - 1))
                            nc.vector.tensor_tensor(
                                x2[qt][:, half * 512:(half + 1) * 512], p2[:],
                                xr[qt][:, half * 512:(half + 1) * 512], op=OP.add)
                        layer_norm_tile(dwork, pspool, x2[qt], h2T, qt * 128)
                        nc.vector.tensor_tensor(x2[qt][:], x2[qt][:], bias2r[:],
                                                op=OP.add)

            # ---- Phase E ----
            with (
                tc.tile_pool(name="phE_a", bufs=1) as e_apool,
                tc.tile_pool(name="phE_w", bufs=4) as e_wpool,
                tc.tile_pool(name="phE_w2", bufs=1) as e_w2pool,
                tc.tile_pool(name="phE", bufs=3) as e_work,
            ):
                aT = [e_apool.tile([128, QS], BF16, tag=f"aT{i}", name=f"aT{i}")
                      for i in range(NFT)]
                w2sb = [e_w2pool.tile([128, D], BF16, tag=f"w2_{i}", name=f"w2_{i}")
                        for i in range(NFT)]
                for ft in range(NFT):
                    nc.sync.dma_start(w2sb[ft][:], d_w2[ft * 128:(ft + 1) * 128, :])
                for ft in range(NFT):
                    w1b = e_wpool.tile([128, D], BF16, tag="w1b", name="w1b")
                    nc.sync.dma_start(
                        w1b[:].rearrange("p (d c) -> p d c", c=128),
                        d_w1t[ft].rearrange("d r c -> r d c"))
                    pa = pspool.tile([128, 512], F32, tag="mm", name="pa", bufs=4)
                    for dt_ in range(NDT):
                        nc.tensor.matmul(
                            pa[:], w1b[:, dt_ * 128:(dt_ + 1) * 128], h2T[dt_][:],
                            start=(dt_ == 0), stop=(dt_ == NDT - 1))
                    nc.scalar.activation(aT[ft][:], pa[:], ACT.Gelu_apprx_tanh,
                                         bias=b1sb[:, ft:ft + 1])
                for qt in range(NQT):
                    ysb = e_work.tile([128, D], F32, tag="ysb", name="ysb")
                    for half in range(2):
                        p3 = pspool.tile([128, 512], F32, tag="acc", name="p3", bufs=2)
                        for ft in range(NFT):
                            nc.tensor.matmul(
                                p3[:], aT[ft][:, qt * 128:(qt + 1) * 128],
                                w2sb[ft][:, half * 512:(half + 1) * 512],
                                start=(ft == 0), stop=(ft == NFT - 1))
                        nc.vector.tensor_tensor(
                            ysb[:, half * 512:(half + 1) * 512], p3[:],
                            x2[qt][:, half * 512:(half + 1) * 512], op=OP.add)
                    nc.sync.dma_start(d_y[qt * 128:(qt + 1) * 128, :], ysb[:])

    nc.compile()
    return nc




def _gelu_tanh(x):
    x = np.asarray(x, np.float64)
    return 0.5 * x * (1.0 + np.tanh(np.sqrt(2.0 / np.pi) * (x + 0.044715 * x ** 3)))


def kernel(x, torus_dist, time_emb, mask, ln1_g, ln1_b, Wqkv, Wout,
           torus_scale, ln2_g, ln2_b, W1, b1, W2, b2, Wt, bt):
    sc_arr = np.asarray(torus_scale, np.float32)
    if not bool(np.all(sc_arr == sc_arr[0])):
        return _kernel_fallback(
            x, torus_dist, time_emb, mask, ln1_g, ln1_b, Wqkv, Wout,
            torus_scale, ln2_g, ln2_b, W1, b1, W2, b2, Wt, bt)

    if "nc_fast" not in _CACHED:
        _CACHED["nc_fast"] = build_fast()
    nc = _CACHED["nc_fast"]

    in_maps = make_inputs(
        x=np.asarray(x), torus_dist=np.asarray(torus_dist),
        time_emb=np.asarray(time_emb), mask=np.asarray(mask),
        ln1_g=np.asarray(ln1_g), ln1_b=np.asarray(ln1_b),
        Wqkv=np.asarray(Wqkv), Wout=np.asarray(Wout),
        torus_scale=sc_arr, ln2_g=np.asarray(ln2_g),
        ln2_b=np.asarray(ln2_b), W1=np.asarray(W1), b1=np.asarray(b1),
        W2=np.asarray(W2), b2=np.asarray(b2), Wt=np.asarray(Wt),
        bt=np.asarray(bt), gelu_tanh=_gelu_tanh)
    res = run_bass_kernel_spmd(nc, in_maps, core_ids=list(range(8)))
    _CACHED["last_results"] = res

    out = np.empty((B, L, D), np.float32)
    for c in range(8):
        b_, qs_ = c // NC_PER_B, c % NC_PER_B
        out[b_, qs_ * QS:(qs_ + 1) * QS, :] = res.results[c]["y"]
    return out


def _kernel_fallback(x, torus_dist, time_emb, mask, ln1_g, ln1_b, Wqkv, Wout,
                     torus_scale, ln2_g, ln2_b, W1, b1, W2, b2, Wt, bt):
    x = np.asarray(x, np.float32)
    torus_dist = np.asarray(torus_dist, np.float32)
    time_emb = np.asarray(time_emb, np.float32)
    mask = np.asarray(mask)
    Wqkv = np.asarray(Wqkv, np.float32)

    sc_arr = np.asarray(torus_scale, np.float32)
    shared = False
    key = "nc_fallback"
    if key not in _CACHED:
        _CACHED[key] = _build_fallback(shared_mask=False)
    nc = _CACHED[key]

    bf = lambda a: np.ascontiguousarray(a).astype(ml_dtypes.bfloat16)
    rep = lambda v: np.ascontiguousarray(
        np.tile(np.asarray(v, np.float32)[None, :], (128, 1)))

    tp = (_gelu_tanh(time_emb) @ np.asarray(Wt, np.float64)
          + np.asarray(bt, np.float64))          # [B, 2D]
    scale, shift = tp[:, :D], tp[:, D:]
    g_eff = (np.asarray(ln1_g, np.float64)[None, :] * (1.0 + scale)).astype(np.float32)
    b_eff = (np.asarray(ln1_b, np.float64)[None, :] * (1.0 + scale) + shift).astype(np.float32)

    Wq_r = np.asarray(Wqkv[:, 0:D], np.float64) / np.sqrt(64.0)
    Wk_r = np.asarray(Wqkv[:, D:2 * D], np.float64)
    Wv_r = np.asarray(Wqkv[:, 2 * D:3 * D], np.float64)
    W1_r = np.asarray(W1, np.float64)
    g2 = np.asarray(ln2_g, np.float64)
    b2ln = np.asarray(ln2_b, np.float64)
    w1t_g = (g2[:, None] * W1_r).astype(np.float32)
    w1t = bf(w1t_g.reshape(8, 128, 32, 128).transpose(2, 0, 1, 3))
    b1sb_eff = (np.asarray(b1, np.float64) + b2ln @ W1_r).astype(np.float32)
    b1sb = np.ascontiguousarray(b1sb_eff.reshape(32, 128).T)
    w2 = bf(W2)
    wout = bf(Wout)
    bias2r = rep(b2)

    in_maps = []
    for c in range(8):
        b_, qs_ = c // NC_PER_B, c % NC_PER_B
        rows = slice(qs_ * QS, (qs_ + 1) * QS)
        km = np.where(mask[b_], 0.0, -88.0).astype(np.float32)      # [L]
        torT = torus_dist[0, rows, :].T.astype(np.float32)           # [L, QS]
        if shared:
            expm = np.exp(km[:, None] - sc_arr[0] * torT).astype(ml_dtypes.bfloat16)
        else:
            expm = np.exp(km[None, :, None] - sc_arr[:, None, None]
                          * torT[None, :, :]).astype(ml_dtypes.bfloat16)
        ge = g_eff[b_].astype(np.float64)
        be = b_eff[b_].astype(np.float64)
        wq_b = bf((ge[:, None] * Wq_r).astype(np.float32))
        wk_b = bf((ge[:, None] * Wk_r).astype(np.float32))
        wv_b = bf((ge[:, None] * Wv_r).astype(np.float32))
        bk = (be @ Wk_r).astype(np.float32)
        bq = (be @ Wq_r).astype(np.float32)
        bv = (be @ Wv_r).astype(np.float32)
        in_maps.append({
            "x_full": x[b_],
            "x_res": np.ascontiguousarray(x[b_, rows]),
            "expm": expm,
            "wq": wq_b, "wk": wk_b, "wv": wv_b, "wout": wout,
            "w1t": w1t, "w2": w2,
            "biask": np.ascontiguousarray(bk.reshape(8, 128).T),
            "biasq": np.ascontiguousarray(bq.reshape(8, 128).T),
            "bvrep": rep(bv),
            "bias2r": bias2r, "b1sb": b1sb,
        })

    import os
    trace = bool(int(os.environ.get("DENOISER_TRACE", "0")))
    res = run_bass_kernel_spmd(nc, in_maps, core_ids=list(range(8)), trace=trace)
    _CACHED["last_results"] = res

    out = np.empty((B, L, D), np.float32)
    for c in range(8):
        b_, qs_ = c // NC_PER_B, c % NC_PER_B
        out[b_, qs_ * QS:(qs_ + 1) * QS, :] = res.results[c]["y"]
    return out

